# revision 39
# baseline (speedup 1.0000x reference)
"""Trainium2 Bass kernel for nn_Attention (dense transformer block).

Data-parallel over batch: 8 batch elements -> 8 NeuronCores, zero collectives.

Per-core dataflow (one batch element, C=256, L=1600, nh=4, dk=32, dh=64):
  1. host: fold all BatchNorms into conv weights/biases; fold softmax scale
     into Wq; permute qkv output channels so the 1x1-conv matmul directly
     emits q/k packed per-head (4 heads x 32 rows) and v head-pairs.
  2. qkv 1x1 conv: matmul (bf16); bias via ScalarE Identity+bias (the two
     prologue tiles use VectorE so they don't wait on the exp-table load).
  3. S^T = k^T q per head, 4 heads row-packed on the PE array (contraction
     32), uniform 400-wide q tiles (4 x 13 chunks, no ragged tail sweep).
     Each head's S^T goes to its OWN one-bank PSUM tile (ring of 4): the
     pipeline pacer is the S^T -> exp -> next-S^T loop per st buffer, and
     one matmul + an FD=400 exp per buffer keeps that loop ~1.0us.
  4. P~^T = exp(S^T), PSUM -> SBUF bf16, split across TWO engines per chunk:
     heads 0,1 on ScalarE (true Exp LUT), heads 2,3 on VectorE via the
     Schraudolph bit-trick: bf16(exp(x)) ~= bitcast_bf16(int16(x*128/ln2
     + 16248.83)) (f32->i16 convert is round-to-nearest-even; max rel err
     ~3%, which dilutes to <2e-3 in the final output since y is a softmax
     average and the conv branches dominate the output norm). One engine
     alone needs ~85-100us for the 10.24M-element exp; the split is the
     main speedup over the original baseline.
  5. y~ = v @ P~^T via col-packed matmuls; softmax sums s = ones^T P~^T via
     col-packed 1-col matmuls, accumulated over k-chunks; both issued TWO
     chunks behind the S^T/exp front so their exp-done waits never
     head-block the in-order PE queue.
  6. y copied out of PSUM immediately at qt end (ScalarE) so the y-bank ring
     frees before the 1/s chain; 1/s via reciprocal_approx_fast straight off
     PSUM, broadcast 4 rows -> 64-partition blocks via one DRAM bounce;
     multiply on DVE, +dw add on GpSimd.
  7. depthwise 3x3 conv on v: 9 diagonal matmuls over a zero-padded copy
     (vpad maintained by GpSimd copies); PSUM->SBUF copy on ScalarE.
  8. final 1x1 conv matmul + bias via ScalarE Identity+bias, DMA out (f32);
     the last two output tiles use the freed y PSUM ring so they pipeline
     instead of serializing on the single misc bank.

All non-attention work (qkv tiles, v^T transposes, depthwise groups, final
conv tiles) is issued at dependency-feasible points INSIDE the attention
k-chunk loop so the in-order PE/ACT queues never stall on cross-phase
chains; y/s matmuls run one k-chunk behind S^T/exp (software pipeline).
"""

import os
import sys

for _p in ("/opt/trn_rl_repo", "/root/.axon_site/_ro/trn_rl_repo"):
    if os.path.isdir(_p) and _p not in sys.path:
        sys.path.insert(0, _p)

import numpy as np
import ml_dtypes

import concourse.bass as bass
import concourse.mybir as mybir
import concourse.tile as tile
from concourse import bacc
from concourse.bass_utils import run_bass_kernel_spmd

BF16 = ml_dtypes.bfloat16
EPS = 1e-3
B, C, H, W = 8, 256, 40, 40
L = H * W            # 1600
NH, DK, DH = 4, 32, 64
C2 = 2 * C           # 512
WP = W + 2           # 42 padded width
LP = (H + 2) * WP    # 1764 padded spatial
KC = [128] * 12 + [64]          # contraction chunks over L
QT = [400, 400, 400, 400]       # uniform q tiles (<=512 PSUM bank)

# Schraudolph bf16 exp: bitcast_bf16(int16(x*128/ln2 + B)); C=0.056 centers
# the log-error (zero mean) so softmax averaging cancels it.
EXP_A = 128.0 / float(np.log(2.0))          # 184.664965...
EXP_B = 16256.0 - 0.056 * 128.0             # 16248.832

# kc chunks where ScalarE takes BOTH exp halves (rebalance knob; the rest
# split half A -> ScalarE, half B -> VectorE).
ACT_BOTH = ()

# qkv tiles whose PSUM->SBUF+bias copy runs on VectorE instead of ScalarE
# (ScalarE carries ~4us more extras than VectorE; this evens the queues).
DVE_QKV = {(2, 2), (3, 2), (0, 3), (1, 3)}

# extra work issued inside the attention loop at (qt, kc); dependencies:
#   qkv(oc,t): x tile t            vT(c,kc'): v tiles up to (kc'*128+127)//400
#   dw(c,t): v tiles t-1..t+1      cb2(oc,t): combines covering cols t*400+400
SCHED = {
    (0, 0): [("qkv", 2, 0), ("qkv", 3, 0), ("vT", 0, 0), ("vT", 1, 0),
             ("qkv", 1, 1)],
    (0, 1): [("qkv", 0, 1), ("qkv", 2, 1), ("qkv", 3, 1),
             ("vT", 0, 1), ("vT", 1, 1), ("vT", 0, 2), ("vT", 1, 2),
             ("vT", 0, 3), ("vT", 1, 3)],
    (0, 2): [("qkv", 0, 2)],
    (0, 3): [("qkv", 1, 2), ("vT", 0, 4), ("vT", 1, 4), ("vT", 0, 5),
             ("vT", 1, 5)],
    (0, 4): [("dw", 0, 0), ("dw", 1, 0)],
    (0, 5): [("qkv", 2, 2), ("qkv", 3, 2), ("vT", 0, 6), ("vT", 1, 6)],
    (0, 6): [("qkv", 0, 3), ("qkv", 1, 3), ("vT", 0, 7), ("vT", 1, 7)],
    (0, 7): [("qkv", 2, 3), ("qkv", 3, 3), ("vT", 0, 8), ("vT", 1, 8)],
    (0, 8): [("dw", 0, 1), ("dw", 1, 1), ("vT", 0, 9), ("vT", 1, 9)],
    (0, 9): [("vT", 0, 10), ("vT", 1, 10), ("vT", 0, 11), ("vT", 1, 11)],
    (0, 10): [("dw", 0, 2), ("dw", 1, 2), ("vT", 0, 12), ("vT", 1, 12)],
    (1, 1): [("dw", 0, 3)],
    (1, 3): [("dw", 1, 3)],
    (1, 5): [("cb2", 0, 0, 400), ("cb2", 1, 0, 400)],
    (2, 2): [("cb2", 0, 400, 400), ("cb2", 1, 400, 400)],
    (3, 1): [("cb2", 0, 800, 400), ("cb2", 1, 800, 400)],
}

_CACHE = {}
LAST_RESULTS = None


def _exp_eng(qt, kc, half):
    if half == 0:
        return "act"
    return "act" if kc in ACT_BOTH else "dve"


def _build_program(nrep=1):
    key = ("nc", nrep)
    if key in _CACHE:
        return _CACHE[key]
    f32 = mybir.dt.float32
    bf16 = mybir.dt.bfloat16
    i16 = mybir.dt.int16
    AF = mybir.ActivationFunctionType
    OP = mybir.AluOpType

    nc = bacc.Bacc("TRN2", target_bir_lowering=False, debug=False)
    x_d = nc.declare_dram_parameter("x", [C, L], bf16, isOutput=False)
    wqkv_d = nc.declare_dram_parameter("wqkv", [128, 8, 128], bf16, isOutput=False)
    bqkv_d = nc.declare_dram_parameter("bqkv", [128, 4], f32, isOutput=False)
    wdiag_d = nc.declare_dram_parameter("wdiag", [128, 18, 128], bf16, isOutput=False)
    wcb2_d = nc.declare_dram_parameter("wcb2", [128, 4, 128], bf16, isOutput=False)
    bcb2_d = nc.declare_dram_parameter("bcb2", [128, 2], f32, isOutput=False)
    out_d = nc.declare_dram_parameter("out", [C, L], f32, isOutput=True)

    trace_sim = os.environ.get("KERNEL_TRACE_SIM", "0") == "1"
    with tile.TileContext(nc, trace_sim=trace_sim) as tc:
        with (
            tc.tile_pool(name="const", bufs=1) as const,
            tc.tile_pool(name="pt", bufs=6) as ptp,
            tc.tile_pool(name="rsml", bufs=3) as rsml,
            tc.tile_pool(name="rb", bufs=6) as rbp,
            tc.tile_pool(name="tmp", bufs=4) as tmpp,
            tc.tile_pool(name="outp", bufs=4) as outp,
            tc.tile_pool(name="st", bufs=4, space="PSUM") as stp,
            tc.tile_pool(name="yps", bufs=2, space="PSUM") as ypp,
            tc.tile_pool(name="sps", bufs=1, space="PSUM") as spp,
            tc.tile_pool(name="misc", bufs=1, space="PSUM") as miscp,
            tc.tile_pool(name="dram", bufs=2, space="DRAM") as dramp,
        ):
            # ---- persistent SBUF tensors ----
            x_sb = const.tile([128, 2, L], bf16)
            wqkv = const.tile([128, 8, 128], bf16)
            bqkv = const.tile([128, 4], f32)
            wdiag = const.tile([128, 18, 128], bf16)
            wcb2 = const.tile([128, 4, 128], bf16)
            bcb2 = const.tile([128, 2], f32)
            qp = const.tile([128, L], bf16)
            kp = const.tile([128, L], bf16)
            vflat = const.tile([128, 2, 1664], bf16)
            vpad = const.tile([128, 2, LP], bf16)
            vT = const.tile([128, 2, 13, 128], bf16)
            dwsb = const.tile([128, 2, L], f32)
            ytot = const.tile([128, 2, L], bf16)
            # all-ones [128,32] lhsT: the s matmuls write 32 redundant rows
            # (M=32 engages a real 128x32 column tile instead of M=1; the
            # 4 s matmuls then occupy 4 distinct col groups and can run
            # concurrently). The recip/broadcast path still reads row 32h.
            ones32 = const.tile([128, 32], bf16)
            dummy = const.tile([1, 1], f32)

            # ---- input DMAs (x tile-major so qkv can start early) ----
            nc.sync.dma_start(out=wqkv[:], in_=wqkv_d.ap())
            for t in range(4):
                for cc in range(2):
                    nc.sync.dma_start(
                        out=x_sb[:, cc, bass.ts(t, 400)],
                        in_=x_d.ap()[cc * 128:(cc + 1) * 128, bass.ts(t, 400)],
                    )
            nc.sync.dma_start(out=bqkv[:], in_=bqkv_d.ap())
            nc.sync.dma_start(out=wdiag[:], in_=wdiag_d.ap())
            nc.sync.dma_start(out=wcb2[:], in_=wcb2_d.ap())
            nc.sync.dma_start(out=bcb2[:], in_=bcb2_d.ap())

            # preload the exp table while DMAs run
            nc.vector.memset(dummy[:], 0.0)
            nc.vector.memset(ones32[:], 1.0)
            nc.scalar.activation(out=dummy[:], in_=dummy[:], func=AF.Exp)

            # zero the pad borders + vflat tail
            nc.gpsimd.memset(vflat[:, :, 1600:1664], 0.0)
            for c in range(2):
                vp3 = vpad[:, c, :].rearrange("p (h w) -> p h w", w=WP)
                nc.gpsimd.memset(vp3[:, 0, :], 0.0)
                nc.gpsimd.memset(vp3[:, 41, :], 0.0)
                nc.gpsimd.memset(vp3[:, 1:41, 0:1], 0.0)
                nc.gpsimd.memset(vp3[:, 1:41, 41:42], 0.0)

            def body(rep):
                def qkv_tile(oc, t, prologue=False):
                    # in-loop qkv must NOT take a y-pool slot: both y slots
                    # are held across each qt sweep and attention depends on
                    # these tiles (deadlock); use the rotating misc bank.
                    pool, tag = (ypp, "y") if prologue else (miscp, "misc")
                    ps = pool.tile([128, 512], f32, tag=tag,
                                   name=f"r{rep}qkvps{oc}_{t}")
                    for cc in range(2):
                        nc.tensor.matmul(
                            ps[:, 0:400],
                            lhsT=wqkv[:, cc * 4 + oc, :],
                            rhs=x_sb[:, cc, bass.ts(t, 400)],
                            start=(cc == 0),
                            stop=(cc == 1),
                        )
                    bias = bqkv[:, oc:oc + 1]
                    if oc == 0:
                        dst = qp[:, bass.ts(t, 400)]
                    elif oc == 1:
                        dst = kp[:, bass.ts(t, 400)]
                    else:
                        dst = vflat[:, oc - 2, bass.ts(t, 400)]
                    if prologue or (oc, t) in DVE_QKV:
                        # DVE is idle at kernel start and, unlike ScalarE,
                        # does not wait on the ~2.7us exp-table load; a few
                        # in-loop tiles also go to DVE for queue balance.
                        nc.vector.tensor_scalar_add(dst, ps[:, 0:400], bias)
                    else:
                        nc.scalar.activation(out=dst, in_=ps[:, 0:400],
                                             func=AF.Identity, bias=bias,
                                             scale=1.0)
                    if oc >= 2:
                        c = oc - 2
                        vp3 = vpad[:, c, :].rearrange("p (h w) -> p h w", w=WP)
                        nc.gpsimd.tensor_copy(
                            vp3[:, 1 + t * 10:11 + t * 10, 1:41],
                            vflat[:, c, bass.ts(t, 400)].rearrange(
                                "p (h w) -> p h w", w=40),
                        )

                def vT_tile(c, kc):
                    # NOTE: do NOT issue transpose DMAs from nc.scalar (the
                    # second HWDGE ring) — it crashes the device at runtime.
                    nc.sync.dma_start(
                        out=vT[:, c, kc, :],
                        in_=vflat[:, c, kc * 128:(kc + 1) * 128],
                        transpose=True,
                    )

                def dw_group(c, t):
                    ps = miscp.tile([128, 512], f32, tag="misc")
                    vp3 = vpad[:, c, :].rearrange("p (h w) -> p h w", w=WP)
                    for tap in range(9):
                        ky, kx = tap // 3, tap % 3
                        nc.tensor.matmul(
                            ps[:, 0:400],
                            lhsT=wdiag[:, tap * 2 + c, :],
                            rhs=vp3[:, ky + t * 10:ky + t * 10 + 10, kx:kx + 40],
                            start=(tap == 0),
                            stop=(tap == 8),
                        )
                    nc.scalar.activation(out=dwsb[:, c, bass.ts(t, 400)],
                                         in_=ps[:, 0:400], func=AF.Copy)

                def cb2_tile(oc, col0, width=400, tail=False):
                    # tail cb2s run after qt3 freed the y banks; using the y
                    # ring instead of the single misc bank lets the final two
                    # tiles pipeline instead of serializing on one PSUM bank.
                    ps = (ypp.tile([128, 512], f32, tag="y",
                                   name=f"r{rep}cb2t{oc}")
                          if tail else miscp.tile([128, 512], f32, tag="misc"))
                    for cc in range(2):
                        nc.tensor.matmul(
                            ps[:, 0:width],
                            lhsT=wcb2[:, cc * 2 + oc, :],
                            rhs=ytot[:, cc, col0:col0 + width],
                            start=(cc == 0),
                            stop=(cc == 1),
                        )
                    ob = outp.tile([128, 400], f32, tag="ob")
                    nc.scalar.activation(out=ob[:, 0:width], in_=ps[:, 0:width],
                                         func=AF.Identity,
                                         bias=bcb2[:, oc:oc + 1], scale=1.0)
                    nc.sync.dma_start(
                        out=out_d.ap()[oc * 128:(oc + 1) * 128,
                                       col0:col0 + width],
                        in_=ob[:, 0:width],
                    )

                def run_action(act):
                    kind = act[0]
                    if kind == "qkv":
                        qkv_tile(act[1], act[2])
                    elif kind == "vT":
                        vT_tile(act[1], act[2])
                    elif kind == "dw":
                        dw_group(act[1], act[2])
                    elif kind == "cb2":
                        cb2_tile(act[1], act[2], act[3])

                def issue_ys(qt, kc, y_ps, s_ps, pt):
                    # y split into M=32 halves and s widened to M=32 so each
                    # round of 4 matmuls covers the 4 distinct 32-col groups
                    # (the PE can run distinct col tiles concurrently; M=64
                    # pairs at 2 positions and M=1 slivers cannot 4-pack).
                    qoff, qlen = qt * 400, QT[qt]
                    krows = KC[kc]
                    for h in range(4):
                        pr, j = h // 2, h % 2
                        for m in range(2):
                            col = 64 * j + 32 * m
                            nc.tensor.matmul(
                                y_ps[pr][col:col + 32, 0:qlen],
                                lhsT=vT[0:krows, pr, kc, col:col + 32],
                                rhs=pt[0:krows, h * qlen:(h + 1) * qlen],
                                start=(kc == 0), stop=(kc == 12),
                                tile_position=(0, col),
                            )
                    for h in range(4):
                        nc.tensor.matmul(
                            s_ps[32 * h:32 * h + 32, 0:qlen],
                            lhsT=ones32[0:krows, 0:32],
                            rhs=pt[0:krows, h * qlen:(h + 1) * qlen],
                            start=(kc == 0), stop=(kc == 12),
                            tile_position=(0, 32 * h),
                        )

                # prologue: the qkv tiles attention immediately needs
                for (oc, t) in [(0, 0), (1, 0)]:
                    qkv_tile(oc, t, prologue=True)

                for qt in range(4):
                    qoff, qlen = qt * 400, QT[qt]
                    y_ps = [ypp.tile([128, 512], f32, tag="y",
                                     name=f"r{rep}y{qt}_{i}") for i in range(2)]
                    s_ps = spp.tile([128, 512], f32, tag="s")
                    pts = {}
                    for kc in range(13):
                        koff, krows = kc * 128, KC[kc]
                        pt = ptp.tile([128, 2048], bf16, tag="pt",
                                      name=f"r{rep}pt{qt}_{kc}")
                        pts[kc] = pt
                        # per-head one-bank st tiles (ring of 4): the pipeline
                        # pacer is the S^T -> exp -> next-S^T loop on each st
                        # buffer; one matmul + an FD=400 exp per buffer makes
                        # that loop ~1.0us instead of ~1.6us. Each head's S^T
                        # still lands in its own PSUM bank (same-bank
                        # concurrent drains crash the device).
                        for h in range(4):
                            st = stp.tile([128, 512], f32, tag="st",
                                          name=f"r{rep}st{qt}_{kc}_{h}")
                            nc.tensor.matmul(
                                st[0:krows, 0:qlen],
                                lhsT=kp[32 * h:32 * h + 32,
                                        koff:koff + krows],
                                rhs=qp[32 * h:32 * h + 32,
                                       qoff:qoff + qlen],
                                start=True, stop=True,
                                tile_position=(32 * h, 0),
                            )
                            out_ap = pt[0:krows, h * qlen:(h + 1) * qlen]
                            in_ap = st[0:krows, 0:qlen]
                            if h < 2:
                                nc.scalar.activation(out=out_ap, in_=in_ap,
                                                     func=AF.Exp)
                            else:
                                nc.vector.tensor_scalar(
                                    out=out_ap.bitcast(i16), in0=in_ap,
                                    scalar1=EXP_A, scalar2=EXP_B,
                                    op0=OP.mult, op1=OP.add)
                        # y/s run FOUR chunks behind so their exp-done
                        # waits never head-block the in-order PE queue
                        if kc > 3:
                            issue_ys(qt, kc - 4, y_ps, s_ps, pts[kc - 4])
                            del pts[kc - 4]
                        for act in SCHED.get((qt, kc), []):
                            run_action(act)
                    for kt in (9, 10, 11, 12):
                        issue_ys(qt, kt, y_ps, s_ps, pts[kt])

                    # copy y out of PSUM immediately (ScalarE) so the y bank
                    # ring frees before the slow 1/s DMA-broadcast chain; the
                    # next qt's y matmuls would otherwise stall ~3us on it.
                    yc = [tmpp.tile([128, 512], f32, tag="yc",
                                    name=f"r{rep}yc{qt}_{i}") for i in range(2)]
                    for c in range(2):
                        nc.scalar.activation(out=yc[c][:, 0:qlen],
                                             in_=y_ps[c][:, 0:qlen],
                                             func=AF.Copy)

                    # softmax normalize + add depthwise branch
                    # (reciprocal_approx_fast: ~18 correct bits, ample for a
                    # softmax denominator; halves the qt-boundary latency)
                    rra = rsml.tile([128, 512], f32, tag="rra")
                    nc.vector.reciprocal_approx_fast(
                        out=rra[:, 0:qlen], in_=s_ps[:, 0:qlen])
                    sden = dramp.tile([4, 512], f32, tag="sden",
                                      name=f"r{rep}sden{qt}")
                    nc.sync.dma_start(
                        out=sden[0:4, 0:qlen],
                        in_=rra[:, 0:qlen].rearrange(
                            "(h r) q -> h r q", r=32)[:, 0, :])
                    rb = [rbp.tile([128, 512], f32, tag="rb",
                                   name=f"r{rep}rb{qt}_{i}") for i in range(2)]
                    for h in range(4):
                        c, j = h // 2, h % 2
                        src_ap = bass.AP(
                            tensor=sden.tensor, offset=sden.offset + h * 512,
                            ap=[[0, 64], [1, qlen]])
                        nc.sync.dma_start(
                            out=rb[c][64 * j:64 * j + 64, 0:qlen], in_=src_ap)
                    for c in range(2):
                        t1 = tmpp.tile([128, 512], f32, tag="t1")
                        nc.vector.tensor_tensor(
                            t1[:, 0:qlen], yc[c][:, 0:qlen],
                            rb[c][:, 0:qlen], op=OP.mult)
                        nc.gpsimd.tensor_tensor(
                            ytot[:, c, qoff:qoff + qlen], t1[:, 0:qlen],
                            dwsb[:, c, qoff:qoff + qlen], op=OP.add)
                # tail: last 400-wide column block depends on qt3's combine
                cb2_tile(0, 1200, 400, tail=True)
                cb2_tile(1, 1200, 400, tail=True)

            for rep in range(nrep):
                body(rep)

    nc.compile()
    _CACHE[key] = nc
    return nc


def _prep_host(x, qkv_w, qkv_g, qkv_b, qkv_m, qkv_v,
               cb1_w, cb1_g, cb1_b, cb1_m, cb1_v,
               cb2_w, cb2_g, cb2_b, cb2_m, cb2_v):
    scale = DK ** -0.5
    inv0 = (qkv_g / np.sqrt(qkv_v + EPS)).astype(np.float64)
    w0 = qkv_w.astype(np.float64) * inv0[:, None]
    b0 = qkv_b.astype(np.float64) - qkv_m.astype(np.float64) * inv0

    qrows = [h * 128 + i for h in range(NH) for i in range(32)]
    krows = [h * 128 + 32 + i for h in range(NH) for i in range(32)]
    vrows = [h * 128 + 64 + i for h in range(NH) for i in range(64)]
    perm = np.array(qrows + krows + vrows)
    w0p, b0p = w0[perm], b0[perm]
    w0p[0:128] *= scale
    b0p[0:128] *= scale

    w0T = np.ascontiguousarray(w0p.T)  # [C, C2]
    wqkv = np.empty((128, 8, 128), dtype=BF16)
    for cc in range(2):
        for oc in range(4):
            wqkv[:, cc * 4 + oc, :] = w0T[cc * 128:(cc + 1) * 128,
                                          oc * 128:(oc + 1) * 128].astype(BF16)
    bqkv = np.ascontiguousarray(
        b0p.reshape(4, 128).T).astype(np.float32)  # [128, 4]

    inv1 = (cb1_g / np.sqrt(cb1_v + EPS)).astype(np.float64)
    w1 = cb1_w[:, 0].astype(np.float64) * inv1[:, None, None]  # [C,3,3]
    b1 = cb1_b.astype(np.float64) - cb1_m.astype(np.float64) * inv1
    wdiag = np.zeros((128, 18, 128), dtype=BF16)
    ar = np.arange(128)
    for tap in range(9):
        ky, kx = tap // 3, tap % 3
        for c in range(2):
            wdiag[ar, tap * 2 + c, ar] = w1[c * 128:(c + 1) * 128,
                                            ky, kx].astype(BF16)

    inv2 = (cb2_g / np.sqrt(cb2_v + EPS)).astype(np.float64)
    w2 = cb2_w.astype(np.float64) * inv2[:, None]
    beta2 = (cb2_b.astype(np.float64) - cb2_m.astype(np.float64) * inv2
             + w2 @ b1)
    w2T = np.ascontiguousarray(w2.T)
    wcb2 = np.empty((128, 4, 128), dtype=BF16)
    for cc in range(2):
        for oc in range(2):
            wcb2[:, cc * 2 + oc, :] = w2T[cc * 128:(cc + 1) * 128,
                                          oc * 128:(oc + 1) * 128].astype(BF16)
    bcb2 = np.ascontiguousarray(
        beta2.reshape(2, 128).T).astype(np.float32)  # [128, 2]

    xbf = np.ascontiguousarray(x.reshape(B, C, L)).astype(BF16)
    return xbf, wqkv, bqkv, wdiag, wcb2, bcb2


def kernel(**inputs):
    global LAST_RESULTS
    inputs = {k: np.asarray(v) for k, v in inputs.items()}
    xbf, wqkv, bqkv, wdiag, wcb2, bcb2 = _prep_host(**inputs)
    nc = _build_program(int(os.environ.get("KERNEL_NREP", "1")))
    in_maps = [
        {"x": xbf[b], "wqkv": wqkv, "bqkv": bqkv,
         "wdiag": wdiag, "wcb2": wcb2, "bcb2": bcb2}
        for b in range(B)
    ]
    res = run_bass_kernel_spmd(nc, in_maps, list(range(8)))
    LAST_RESULTS = res
    out = np.stack([res.results[b]["out"] for b in range(B)])
    return out.reshape(B, C, H, W).astype(np.float32)



# revision 41
# speedup vs baseline: 1.0236x; 1.0236x over previous
"""Trainium2 Bass kernel for nn_Attention (dense transformer block).

Data-parallel over batch: 8 batch elements -> 8 NeuronCores, zero collectives.

Per-core dataflow (one batch element, C=256, L=1600, nh=4, dk=32, dh=64):
  1. host: fold all BatchNorms into conv weights/biases; fold softmax scale
     into Wq; permute qkv output channels so the 1x1-conv matmul directly
     emits q/k packed per-head (4 heads x 32 rows) and v head-pairs.
  2. qkv 1x1 conv: matmul (bf16); bias via ScalarE Identity+bias (the two
     prologue tiles use VectorE so they don't wait on the exp-table load).
  3. S^T = k^T q per head, 4 heads row-packed on the PE array (contraction
     32), uniform 400-wide q tiles (4 x 13 chunks, no ragged tail sweep).
     Each head's S^T goes to its OWN one-bank PSUM tile (ring of 4): the
     pipeline pacer is the S^T -> exp -> next-S^T loop per st buffer, and
     one matmul + an FD=400 exp per buffer keeps that loop ~1.0us.
  4. P~^T = exp(S^T), PSUM -> SBUF bf16, split across TWO engines per chunk:
     heads 0,1 on ScalarE (true Exp LUT), heads 2,3 on VectorE via the
     Schraudolph bit-trick: bf16(exp(x)) ~= bitcast_bf16(int16(x*128/ln2
     + 16248.83)) (f32->i16 convert is round-to-nearest-even; max rel err
     ~3%, which dilutes to <2e-3 in the final output since y is a softmax
     average and the conv branches dominate the output norm). One engine
     alone needs ~85-100us for the 10.24M-element exp; the split is the
     main speedup over the original baseline.
  5. y~ = v @ P~^T via col-packed matmuls; softmax sums s = ones^T P~^T via
     col-packed 1-col matmuls, accumulated over k-chunks; both issued TWO
     chunks behind the S^T/exp front so their exp-done waits never
     head-block the in-order PE queue.
  6. y copied out of PSUM immediately at qt end (ScalarE) so the y-bank ring
     frees before the 1/s chain; 1/s via reciprocal_approx_fast straight off
     PSUM, broadcast 4 rows -> 64-partition blocks via one DRAM bounce;
     multiply on DVE, +dw add on GpSimd.
  7. depthwise 3x3 conv on v: 9 diagonal matmuls over a zero-padded copy
     (vpad maintained by GpSimd copies); PSUM->SBUF copy on ScalarE.
  8. final 1x1 conv matmul + bias via ScalarE Identity+bias, DMA out (f32);
     the last two output tiles use the freed y PSUM ring so they pipeline
     instead of serializing on the single misc bank.

All non-attention work (qkv tiles, v^T transposes, depthwise groups, final
conv tiles) is issued at dependency-feasible points INSIDE the attention
k-chunk loop so the in-order PE/ACT queues never stall on cross-phase
chains; y/s matmuls run one k-chunk behind S^T/exp (software pipeline).
"""

import os
import sys

for _p in ("/opt/trn_rl_repo", "/root/.axon_site/_ro/trn_rl_repo"):
    if os.path.isdir(_p) and _p not in sys.path:
        sys.path.insert(0, _p)

import numpy as np
import ml_dtypes

import concourse.bass as bass
import concourse.mybir as mybir
import concourse.tile as tile
from concourse import bacc
from concourse.bass_utils import run_bass_kernel_spmd

BF16 = ml_dtypes.bfloat16
EPS = 1e-3
B, C, H, W = 8, 256, 40, 40
L = H * W            # 1600
NH, DK, DH = 4, 32, 64
C2 = 2 * C           # 512
WP = W + 2           # 42 padded width
LP = (H + 2) * WP    # 1764 padded spatial
KC = [128] * 12 + [64]          # contraction chunks over L
QT = [400, 400, 400, 400]       # uniform q tiles (<=512 PSUM bank)

# Schraudolph bf16 exp: bitcast_bf16(int16(x*128/ln2 + B)); C=0.056 centers
# the log-error (zero mean) so softmax averaging cancels it.
EXP_A = 128.0 / float(np.log(2.0))          # 184.664965...
EXP_B = 16256.0 - 0.056 * 128.0             # 16248.832

# kc chunks where ScalarE takes BOTH exp halves (rebalance knob; the rest
# split half A -> ScalarE, half B -> VectorE).
ACT_BOTH = ()

# qkv tiles whose PSUM->SBUF+bias copy runs on VectorE instead of ScalarE
# (ScalarE carries ~4us more extras than VectorE; this evens the queues).
DVE_QKV = {(2, 2), (3, 2), (0, 3), (1, 3)}

# extra work issued inside the attention loop at (qt, kc); dependencies:
#   qkv(oc,t): x tile t            vT(c,kc'): v tiles up to (kc'*128+127)//400
#   dw(c,t): v tiles t-1..t+1      cb2(oc,t): combines covering cols t*400+400
SCHED = {
    (0, 0): [("qkv", 2, 0), ("qkv", 3, 0), ("vT", 0, 0), ("vT", 1, 0),
             ("qkv", 1, 1)],
    (0, 1): [("qkv", 0, 1), ("qkv", 2, 1), ("qkv", 3, 1),
             ("vT", 0, 1), ("vT", 1, 1), ("vT", 0, 2), ("vT", 1, 2),
             ("vT", 0, 3), ("vT", 1, 3)],
    (0, 2): [("qkv", 0, 2)],
    (0, 3): [("qkv", 1, 2), ("vT", 0, 4), ("vT", 1, 4), ("vT", 0, 5),
             ("vT", 1, 5)],
    (0, 4): [("dw", 0, 0), ("dw", 1, 0)],
    (0, 5): [("qkv", 2, 2), ("qkv", 3, 2), ("vT", 0, 6), ("vT", 1, 6)],
    (0, 6): [("qkv", 0, 3), ("qkv", 1, 3), ("vT", 0, 7), ("vT", 1, 7)],
    (0, 7): [("qkv", 2, 3), ("qkv", 3, 3), ("vT", 0, 8), ("vT", 1, 8)],
    (0, 8): [("dw", 0, 1), ("dw", 1, 1), ("vT", 0, 9), ("vT", 1, 9)],
    (0, 9): [("vT", 0, 10), ("vT", 1, 10), ("vT", 0, 11), ("vT", 1, 11)],
    (0, 10): [("dw", 0, 2), ("dw", 1, 2), ("vT", 0, 12), ("vT", 1, 12)],
    (1, 1): [("dw", 0, 3)],
    (1, 3): [("dw", 1, 3)],
    (1, 5): [("cb2", 0, 0, 400), ("cb2", 1, 0, 400)],
    (2, 2): [("cb2", 0, 400, 400), ("cb2", 1, 400, 400)],
    (3, 1): [("cb2", 0, 800, 400), ("cb2", 1, 800, 400)],
}

_CACHE = {}
LAST_RESULTS = None


def _exp_eng(qt, kc, half):
    if half == 0:
        return "act"
    return "act" if kc in ACT_BOTH else "dve"


def _build_program(nrep=1):
    key = ("nc", nrep)
    if key in _CACHE:
        return _CACHE[key]
    f32 = mybir.dt.float32
    bf16 = mybir.dt.bfloat16
    i16 = mybir.dt.int16
    AF = mybir.ActivationFunctionType
    OP = mybir.AluOpType

    nc = bacc.Bacc("TRN2", target_bir_lowering=False, debug=False)
    x_d = nc.declare_dram_parameter("x", [C, L], bf16, isOutput=False)
    wqkv_d = nc.declare_dram_parameter("wqkv", [128, 8, 128], bf16, isOutput=False)
    bqkv_d = nc.declare_dram_parameter("bqkv", [128, 4], f32, isOutput=False)
    wdiag_d = nc.declare_dram_parameter("wdiag", [128, 18, 128], bf16, isOutput=False)
    wcb2_d = nc.declare_dram_parameter("wcb2", [128, 4, 128], bf16, isOutput=False)
    bcb2_d = nc.declare_dram_parameter("bcb2", [128, 2], f32, isOutput=False)
    out_d = nc.declare_dram_parameter("out", [C, L], f32, isOutput=True)

    trace_sim = os.environ.get("KERNEL_TRACE_SIM", "0") == "1"
    with tile.TileContext(nc, trace_sim=trace_sim) as tc:
        with (
            tc.tile_pool(name="const", bufs=1) as const,
            tc.tile_pool(name="pt", bufs=6) as ptp,
            tc.tile_pool(name="rsml", bufs=3) as rsml,
            tc.tile_pool(name="rb", bufs=6) as rbp,
            tc.tile_pool(name="tmp", bufs=4) as tmpp,
            tc.tile_pool(name="outp", bufs=4) as outp,
            tc.tile_pool(name="st", bufs=4, space="PSUM") as stp,
            tc.tile_pool(name="yps", bufs=2, space="PSUM") as ypp,
            tc.tile_pool(name="sps", bufs=1, space="PSUM") as spp,
            tc.tile_pool(name="misc", bufs=1, space="PSUM") as miscp,
            tc.tile_pool(name="dram", bufs=2, space="DRAM") as dramp,
        ):
            # ---- persistent SBUF tensors ----
            x_sb = const.tile([128, 2, L], bf16)
            wqkv = const.tile([128, 8, 128], bf16)
            bqkv = const.tile([128, 4], f32)
            wdiag = const.tile([128, 18, 128], bf16)
            wcb2 = const.tile([128, 4, 128], bf16)
            bcb2 = const.tile([128, 2], f32)
            qp = const.tile([128, L], bf16)
            kp = const.tile([128, L], bf16)
            vflat = const.tile([128, 2, 1664], bf16)
            vpad = const.tile([128, 2, LP], bf16)
            vT = const.tile([128, 2, 13, 128], bf16)
            dwsb = const.tile([128, 2, L], f32)
            ytot = const.tile([128, 2, L], bf16)
            # all-ones [128,32] lhsT: the s matmuls write 32 redundant rows
            # (M=32 engages a real 128x32 column tile instead of M=1; the
            # 4 s matmuls then occupy 4 distinct col groups and can run
            # concurrently). The recip/broadcast path still reads row 32h.
            ones32 = const.tile([128, 32], bf16)
            dummy = const.tile([1, 1], f32)

            # ---- input DMAs (x tile-major so qkv can start early) ----
            nc.sync.dma_start(out=wqkv[:], in_=wqkv_d.ap())
            for t in range(4):
                for cc in range(2):
                    nc.sync.dma_start(
                        out=x_sb[:, cc, bass.ts(t, 400)],
                        in_=x_d.ap()[cc * 128:(cc + 1) * 128, bass.ts(t, 400)],
                    )
            nc.sync.dma_start(out=bqkv[:], in_=bqkv_d.ap())
            nc.sync.dma_start(out=wdiag[:], in_=wdiag_d.ap())
            nc.sync.dma_start(out=wcb2[:], in_=wcb2_d.ap())
            nc.sync.dma_start(out=bcb2[:], in_=bcb2_d.ap())

            # preload the exp table while DMAs run
            nc.vector.memset(dummy[:], 0.0)
            nc.vector.memset(ones32[:], 1.0)
            nc.scalar.activation(out=dummy[:], in_=dummy[:], func=AF.Exp)

            # zero the pad borders + vflat tail
            nc.gpsimd.memset(vflat[:, :, 1600:1664], 0.0)
            for c in range(2):
                vp3 = vpad[:, c, :].rearrange("p (h w) -> p h w", w=WP)
                nc.gpsimd.memset(vp3[:, 0, :], 0.0)
                nc.gpsimd.memset(vp3[:, 41, :], 0.0)
                nc.gpsimd.memset(vp3[:, 1:41, 0:1], 0.0)
                nc.gpsimd.memset(vp3[:, 1:41, 41:42], 0.0)

            def body(rep):
                def qkv_tile(oc, t, prologue=False):
                    # in-loop qkv must NOT take a y-pool slot: both y slots
                    # are held across each qt sweep and attention depends on
                    # these tiles (deadlock); use the rotating misc bank.
                    pool, tag = (ypp, "y") if prologue else (miscp, "misc")
                    ps = pool.tile([128, 512], f32, tag=tag,
                                   name=f"r{rep}qkvps{oc}_{t}")
                    for cc in range(2):
                        nc.tensor.matmul(
                            ps[:, 0:400],
                            lhsT=wqkv[:, cc * 4 + oc, :],
                            rhs=x_sb[:, cc, bass.ts(t, 400)],
                            start=(cc == 0),
                            stop=(cc == 1),
                        )
                    bias = bqkv[:, oc:oc + 1]
                    if oc == 0:
                        dst = qp[:, bass.ts(t, 400)]
                    elif oc == 1:
                        dst = kp[:, bass.ts(t, 400)]
                    else:
                        dst = vflat[:, oc - 2, bass.ts(t, 400)]
                    if prologue or (oc, t) in DVE_QKV:
                        # DVE is idle at kernel start and, unlike ScalarE,
                        # does not wait on the ~2.7us exp-table load; a few
                        # in-loop tiles also go to DVE for queue balance.
                        nc.vector.tensor_scalar_add(dst, ps[:, 0:400], bias)
                    else:
                        nc.scalar.activation(out=dst, in_=ps[:, 0:400],
                                             func=AF.Identity, bias=bias,
                                             scale=1.0)
                    if oc >= 2:
                        c = oc - 2
                        vp3 = vpad[:, c, :].rearrange("p (h w) -> p h w", w=WP)
                        nc.gpsimd.tensor_copy(
                            vp3[:, 1 + t * 10:11 + t * 10, 1:41],
                            vflat[:, c, bass.ts(t, 400)].rearrange(
                                "p (h w) -> p h w", w=40),
                        )

                def vT_tile(c, kc):
                    # NOTE: do NOT issue transpose DMAs from nc.scalar (the
                    # second HWDGE ring) — it crashes the device at runtime.
                    nc.sync.dma_start(
                        out=vT[:, c, kc, :],
                        in_=vflat[:, c, kc * 128:(kc + 1) * 128],
                        transpose=True,
                    )

                def dw_group(c, t):
                    ps = miscp.tile([128, 512], f32, tag="misc")
                    vp3 = vpad[:, c, :].rearrange("p (h w) -> p h w", w=WP)
                    for tap in range(9):
                        ky, kx = tap // 3, tap % 3
                        nc.tensor.matmul(
                            ps[:, 0:400],
                            lhsT=wdiag[:, tap * 2 + c, :],
                            rhs=vp3[:, ky + t * 10:ky + t * 10 + 10, kx:kx + 40],
                            start=(tap == 0),
                            stop=(tap == 8),
                        )
                    nc.scalar.activation(out=dwsb[:, c, bass.ts(t, 400)],
                                         in_=ps[:, 0:400], func=AF.Copy)

                def cb2_tile(oc, col0, width=400, tail=False):
                    # tail cb2s run after qt3 freed the y banks; using the y
                    # ring instead of the single misc bank lets the final two
                    # tiles pipeline instead of serializing on one PSUM bank.
                    ps = (ypp.tile([128, 512], f32, tag="y",
                                   name=f"r{rep}cb2t{oc}")
                          if tail else miscp.tile([128, 512], f32, tag="misc"))
                    for cc in range(2):
                        nc.tensor.matmul(
                            ps[:, 0:width],
                            lhsT=wcb2[:, cc * 2 + oc, :],
                            rhs=ytot[:, cc, col0:col0 + width],
                            start=(cc == 0),
                            stop=(cc == 1),
                        )
                    ob = outp.tile([128, 400], f32, tag="ob")
                    nc.scalar.activation(out=ob[:, 0:width], in_=ps[:, 0:width],
                                         func=AF.Identity,
                                         bias=bcb2[:, oc:oc + 1], scale=1.0)
                    nc.sync.dma_start(
                        out=out_d.ap()[oc * 128:(oc + 1) * 128,
                                       col0:col0 + width],
                        in_=ob[:, 0:width],
                    )

                def run_action(act):
                    kind = act[0]
                    if kind == "qkv":
                        qkv_tile(act[1], act[2])
                    elif kind == "vT":
                        vT_tile(act[1], act[2])
                    elif kind == "dw":
                        dw_group(act[1], act[2])
                    elif kind == "cb2":
                        cb2_tile(act[1], act[2], act[3])

                def issue_ys(qt, kc, y_ps, s_ps, pt):
                    # y split into M=32 halves and s widened to M=32 so each
                    # round of 4 matmuls covers the 4 distinct 32-col groups
                    # (the PE can run distinct col tiles concurrently; M=64
                    # pairs at 2 positions and M=1 slivers cannot 4-pack).
                    qoff, qlen = qt * 400, QT[qt]
                    krows = KC[kc]
                    for h in range(4):
                        pr, j = h // 2, h % 2
                        for m in range(2):
                            col = 64 * j + 32 * m
                            nc.tensor.matmul(
                                y_ps[pr][col:col + 32, 0:qlen],
                                lhsT=vT[0:krows, pr, kc, col:col + 32],
                                rhs=pt[0:krows, h * qlen:(h + 1) * qlen],
                                start=(kc == 0), stop=(kc == 12),
                                tile_position=(0, col),
                            )
                    for h in range(4):
                        nc.tensor.matmul(
                            s_ps[32 * h:32 * h + 32, 0:qlen],
                            lhsT=ones32[0:krows, 0:32],
                            rhs=pt[0:krows, h * qlen:(h + 1) * qlen],
                            start=(kc == 0), stop=(kc == 12),
                            tile_position=(0, 32 * h),
                        )

                # prologue: the qkv tiles attention immediately needs
                for (oc, t) in [(0, 0), (1, 0)]:
                    qkv_tile(oc, t, prologue=True)

                for qt in range(4):
                    qoff, qlen = qt * 400, QT[qt]
                    y_ps = [ypp.tile([128, 512], f32, tag="y",
                                     name=f"r{rep}y{qt}_{i}") for i in range(2)]
                    s_ps = spp.tile([128, 512], f32, tag="s")
                    pts = {}
                    for kc in range(13):
                        koff, krows = kc * 128, KC[kc]
                        pt = ptp.tile([128, 2048], bf16, tag="pt",
                                      name=f"r{rep}pt{qt}_{kc}")
                        pts[kc] = pt
                        # per-head one-bank st tiles (ring of 4): the pipeline
                        # pacer is the S^T -> exp -> next-S^T loop on each st
                        # buffer; one matmul + an FD=400 exp per buffer makes
                        # that loop ~1.0us instead of ~1.6us. Each head's S^T
                        # still lands in its own PSUM bank (same-bank
                        # concurrent drains crash the device).
                        for h in range(4):
                            st = stp.tile([128, 512], f32, tag="st",
                                          name=f"r{rep}st{qt}_{kc}_{h}")
                            nc.tensor.matmul(
                                st[0:krows, 0:qlen],
                                lhsT=kp[32 * h:32 * h + 32,
                                        koff:koff + krows],
                                rhs=qp[32 * h:32 * h + 32,
                                       qoff:qoff + qlen],
                                start=True, stop=True,
                                tile_position=(32 * h, 0),
                            )
                            out_ap = pt[0:krows, h * qlen:(h + 1) * qlen]
                            in_ap = st[0:krows, 0:qlen]
                            if h < 2:
                                nc.scalar.activation(out=out_ap, in_=in_ap,
                                                     func=AF.Exp)
                            else:
                                nc.vector.tensor_scalar(
                                    out=out_ap.bitcast(i16), in0=in_ap,
                                    scalar1=EXP_A, scalar2=EXP_B,
                                    op0=OP.mult, op1=OP.add)
                        # y/s run THREE chunks behind so their exp-done
                        # waits never head-block the in-order PE queue
                        if kc > 2:
                            issue_ys(qt, kc - 3, y_ps, s_ps, pts[kc - 3])
                            del pts[kc - 3]
                        for act in SCHED.get((qt, kc), []):
                            run_action(act)
                    issue_ys(qt, 10, y_ps, s_ps, pts[10])
                    issue_ys(qt, 11, y_ps, s_ps, pts[11])
                    issue_ys(qt, 12, y_ps, s_ps, pts[12])

                    # copy y out of PSUM immediately (ScalarE) so the y bank
                    # ring frees before the slow 1/s DMA-broadcast chain; the
                    # next qt's y matmuls would otherwise stall ~3us on it.
                    yc = [tmpp.tile([128, 512], f32, tag="yc",
                                    name=f"r{rep}yc{qt}_{i}") for i in range(2)]
                    for c in range(2):
                        nc.scalar.activation(out=yc[c][:, 0:qlen],
                                             in_=y_ps[c][:, 0:qlen],
                                             func=AF.Copy)

                    # softmax normalize + add depthwise branch
                    # (reciprocal_approx_fast: ~18 correct bits, ample for a
                    # softmax denominator; halves the qt-boundary latency)
                    rra = rsml.tile([128, 512], f32, tag="rra")
                    nc.vector.reciprocal_approx_fast(
                        out=rra[:, 0:qlen], in_=s_ps[:, 0:qlen])
                    sden = dramp.tile([4, 512], f32, tag="sden",
                                      name=f"r{rep}sden{qt}")
                    nc.sync.dma_start(
                        out=sden[0:4, 0:qlen],
                        in_=rra[:, 0:qlen].rearrange(
                            "(h r) q -> h r q", r=32)[:, 0, :])
                    rb = [rbp.tile([128, 512], f32, tag="rb",
                                   name=f"r{rep}rb{qt}_{i}") for i in range(2)]
                    for h in range(4):
                        c, j = h // 2, h % 2
                        src_ap = bass.AP(
                            tensor=sden.tensor, offset=sden.offset + h * 512,
                            ap=[[0, 64], [1, qlen]])
                        nc.sync.dma_start(
                            out=rb[c][64 * j:64 * j + 64, 0:qlen], in_=src_ap)
                    for c in range(2):
                        # both normalize ops on GpSimd (SBUF-only, ~60%
                        # idle): keeps the 1/s multiply off VectorE, which
                        # is the heavier exp engine after DVE_QKV.
                        t1 = tmpp.tile([128, 512], f32, tag="t1")
                        nc.gpsimd.tensor_tensor(
                            t1[:, 0:qlen], yc[c][:, 0:qlen],
                            rb[c][:, 0:qlen], op=OP.mult)
                        nc.gpsimd.tensor_tensor(
                            ytot[:, c, qoff:qoff + qlen], t1[:, 0:qlen],
                            dwsb[:, c, qoff:qoff + qlen], op=OP.add)
                # tail: last 400-wide column block depends on qt3's combine
                cb2_tile(0, 1200, 400, tail=True)
                cb2_tile(1, 1200, 400, tail=True)

            for rep in range(nrep):
                body(rep)

    nc.compile()
    _CACHE[key] = nc
    return nc


def _prep_host(x, qkv_w, qkv_g, qkv_b, qkv_m, qkv_v,
               cb1_w, cb1_g, cb1_b, cb1_m, cb1_v,
               cb2_w, cb2_g, cb2_b, cb2_m, cb2_v):
    scale = DK ** -0.5
    inv0 = (qkv_g / np.sqrt(qkv_v + EPS)).astype(np.float64)
    w0 = qkv_w.astype(np.float64) * inv0[:, None]
    b0 = qkv_b.astype(np.float64) - qkv_m.astype(np.float64) * inv0

    qrows = [h * 128 + i for h in range(NH) for i in range(32)]
    krows = [h * 128 + 32 + i for h in range(NH) for i in range(32)]
    vrows = [h * 128 + 64 + i for h in range(NH) for i in range(64)]
    perm = np.array(qrows + krows + vrows)
    w0p, b0p = w0[perm], b0[perm]
    w0p[0:128] *= scale
    b0p[0:128] *= scale

    w0T = np.ascontiguousarray(w0p.T)  # [C, C2]
    wqkv = np.empty((128, 8, 128), dtype=BF16)
    for cc in range(2):
        for oc in range(4):
            wqkv[:, cc * 4 + oc, :] = w0T[cc * 128:(cc + 1) * 128,
                                          oc * 128:(oc + 1) * 128].astype(BF16)
    bqkv = np.ascontiguousarray(
        b0p.reshape(4, 128).T).astype(np.float32)  # [128, 4]

    inv1 = (cb1_g / np.sqrt(cb1_v + EPS)).astype(np.float64)
    w1 = cb1_w[:, 0].astype(np.float64) * inv1[:, None, None]  # [C,3,3]
    b1 = cb1_b.astype(np.float64) - cb1_m.astype(np.float64) * inv1
    wdiag = np.zeros((128, 18, 128), dtype=BF16)
    ar = np.arange(128)
    for tap in range(9):
        ky, kx = tap // 3, tap % 3
        for c in range(2):
            wdiag[ar, tap * 2 + c, ar] = w1[c * 128:(c + 1) * 128,
                                            ky, kx].astype(BF16)

    inv2 = (cb2_g / np.sqrt(cb2_v + EPS)).astype(np.float64)
    w2 = cb2_w.astype(np.float64) * inv2[:, None]
    beta2 = (cb2_b.astype(np.float64) - cb2_m.astype(np.float64) * inv2
             + w2 @ b1)
    w2T = np.ascontiguousarray(w2.T)
    wcb2 = np.empty((128, 4, 128), dtype=BF16)
    for cc in range(2):
        for oc in range(2):
            wcb2[:, cc * 2 + oc, :] = w2T[cc * 128:(cc + 1) * 128,
                                          oc * 128:(oc + 1) * 128].astype(BF16)
    bcb2 = np.ascontiguousarray(
        beta2.reshape(2, 128).T).astype(np.float32)  # [128, 2]

    xbf = np.ascontiguousarray(x.reshape(B, C, L)).astype(BF16)
    return xbf, wqkv, bqkv, wdiag, wcb2, bcb2


def kernel(**inputs):
    global LAST_RESULTS
    inputs = {k: np.asarray(v) for k, v in inputs.items()}
    xbf, wqkv, bqkv, wdiag, wcb2, bcb2 = _prep_host(**inputs)
    nc = _build_program(int(os.environ.get("KERNEL_NREP", "1")))
    in_maps = [
        {"x": xbf[b], "wqkv": wqkv, "bqkv": bqkv,
         "wdiag": wdiag, "wcb2": wcb2, "bcb2": bcb2}
        for b in range(B)
    ]
    res = run_bass_kernel_spmd(nc, in_maps, list(range(8)))
    LAST_RESULTS = res
    out = np.stack([res.results[b]["out"] for b in range(B)])
    return out.reshape(B, C, H, W).astype(np.float32)

